# revision 12
# baseline (speedup 1.0000x reference)
"""DarkChannelPrior kernel for 8 Trainium2 NeuronCores.

Computes: dark = min over 3 channels of images [16,3,1024,1024], then a
15x15 box-average convolution (padding 7) -> [16,1,1024,1024].

The kernel is HBM-bandwidth bound (measured ~48 GB/s/core effective with all
8 cores streaming), so the layout minimizes HBM traffic:
  - Pure data parallel: 2 images per core across 8 cores.
  - Inputs are loaded FLAT (no halo duplication): 6 x 2MB HWDGE DMAs per
    image, all on the single qSP queue (multi-queue and SWDGE measured
    slower).  25.2MB/core input traffic, the minimum.
  - Channel min on VectorE (f32 in, bf16 out), into a flat dark tile
    [128, 8 row-blocks x 1024].
  - The separable 15x15 box filter runs as two banded-matmul passes on
    TensorE over aligned 128-row blocks.  Per (block, 128-output-block):
    one K=128 banded matmul plus K=7 edge matmuls against the neighbor
    blocks' 7 boundary rows (N=7 output columns each), replacing the old
    overlapped-tile layout that cost +14% input DMA traffic.
  - Pass-2 PSUM results are scaled and cast to bf16 on ScalarE and stored
    per 128-row block (256KB ascending DMAs overlapping pass-2 compute).
    Output traffic is halved vs f32; the host upcasts to f32 --
    quantization adds ~1e-3 relative error, well under the gate.

Total HBM traffic: 25.2MB in + 2.1MB x 2 out = 29.4MB/core, vs 37.1MB for
the previous overlapped-tile f32 version (halo re-reads eliminated by K=64/
K=7 edge matmuls at PE-legal base partitions 0/64).
"""

import numpy as np
import ml_dtypes

import concourse.bacc as bacc
import concourse.bass as bass
import concourse.tile as tile
import concourse.mybir as mybir
from concourse.bass_utils import run_bass_kernel_spmd

KS = 15
PAD = KS // 2
H = W = 1024
IMGS_PER_CORE = 2
N_CORES = 8
CH = 3
NB = 8  # 128-row blocks per image

LAST_RESULTS = None
_PROGRAM_CACHE = {}


def _build_bmat():
    """Band matrices as one [128, 144] bf16 tensor.

    cols 0:128    B_main[k, j] = 1 iff |k-j| <= 7          (partitions 0:128)
    cols 128:136  B_top[p, j]  = 1 iff j <= p-121, j<=6    (partitions 121:128)
                  -- contribution of the PREVIOUS block's last 7 rows
    cols 136:144  B_bot[p, j'] = 1 iff j' >= p, j'<=6      (partitions 0:7)
                  -- contribution of the NEXT block's first 7 rows, output
                  columns j = 121 + j'
    """
    B = np.zeros((128, 144), dtype=np.float32)
    k = np.arange(128)[:, None]
    j = np.arange(128)[None, :]
    B[:, 0:128] = (np.abs(k - j) <= PAD).astype(np.float32)
    for p in range(121, 128):
        for jj in range(0, p - 121 + 1):
            B[p, 128 + jj] = 1.0
    for p in range(0, 7):
        for jj in range(p, 7):
            B[p, 136 + jj] = 1.0
    return B.astype(ml_dtypes.bfloat16)


def _boxsum_pass(nc, pspool, src_lhsT, dst_writer, bmat):
    """One banded pass: for each 128-wide output block `ob` of the transposed
    orientation, accumulate the 15-tap partition-axis sums of all 8 source
    blocks into two [128,512] PSUM banks, then hand them to
    dst_writer(bank_idx, psum_ap, ob_unused, ob).

    src_lhsT(ob, sb, p0, p1) -> lhsT AP covering source-block sb's
    partitions [p0,p1) for output block ob (free dim = the 128 columns of
    ob's orientation).
    """
    for ob in range(NB):
        psA = pspool.tile([128, 512], mybir.dt.float32, tag="ps")
        psB = pspool.tile([128, 512], mybir.dt.float32, tag="ps")
        for sb in range(NB):
            ps, off = (psA, sb * 128) if sb < 4 else (psB, (sb - 4) * 128)
            has_top = sb > 0
            has_bot = sb < NB - 1
            nc.tensor.matmul(
                ps[:, off : off + 128],
                lhsT=src_lhsT(ob, sb, 0, 128),
                rhs=bmat[0:128, 0:128],
                start=True,
                stop=not (has_top or has_bot),
            )
            if has_top:
                # PE operands must start at partition 0/32/64: take K=64 from
                # base 64 of the previous block (band rows 64:121 are zero).
                nc.tensor.matmul(
                    ps[:, off : off + 7],
                    lhsT=src_lhsT(ob, sb - 1, 64, 128),
                    rhs=bmat[64:128, 128:135],
                    start=False,
                    stop=not has_bot,
                    skip_group_check=True,
                )
            if has_bot:
                nc.tensor.matmul(
                    ps[:, off + 121 : off + 128],
                    lhsT=src_lhsT(ob, sb + 1, 0, 7),
                    rhs=bmat[0:7, 136:143],
                    start=False,
                    stop=True,
                    skip_group_check=True,
                )
        dst_writer(0, psA, ob)
        dst_writer(1, psB, ob)


def _build_program(scale, reps=1, mode="full"):
    # Bacc (not raw Bass): its compile() pipeline splits multi-wait
    # instructions via event semaphores, which TRN2 walrus codegen requires.
    nc = bacc.Bacc(
        "TRN2", target_bir_lowering=False, debug=False, num_devices=N_CORES
    )
    x = nc.dram_tensor(
        "x", [IMGS_PER_CORE, 3, H, W], mybir.dt.float32, kind="ExternalInput"
    )
    bm = nc.dram_tensor("bmat", [128, 144], mybir.dt.bfloat16, kind="ExternalInput")
    y = nc.dram_tensor(
        "y", [IMGS_PER_CORE, H, W], mybir.dt.bfloat16, kind="ExternalOutput"
    )

    with tile.TileContext(nc) as tc:
        with (
            tc.tile_pool(name="const", bufs=1) as cpool,
            tc.tile_pool(name="chan", bufs=2) as chpool,
            tc.tile_pool(name="dark", bufs=2) as dpool,
            tc.tile_pool(name="t1", bufs=1) as t1pool,
            tc.tile_pool(name="outp", bufs=2) as opool,
            tc.tile_pool(name="psum", bufs=8, space="PSUM") as pspool,
        ):
            bmat = cpool.tile([128, 144], mybir.dt.bfloat16)
            nc.sync.dma_start(bmat[:], bm[:])

            import contextlib

            loop_cm = tc.For_i(0, reps, 1) if reps > 1 else contextlib.nullcontext()
            with loop_cm:
                _emit_images(
                    nc, tc, x, y, bmat, scale, chpool, dpool, t1pool, opool,
                    pspool, mode,
                )
    nc.compile()
    return nc


def _emit_images(
    nc, tc, x, y, bmat, scale, chpool, dpool, t1pool, opool, pspool, mode="full"
):
    do_in = mode in ("full", "dma")
    do_compute = mode == "full"
    do_out = mode in ("full", "dma")
    HB = NB // 2  # row-blocks per half-image chunk

    for i in range(IMGS_PER_CORE):
        dark = dpool.tile([128, NB * W], mybir.dt.bfloat16, tag="dark",
                          name=f"dark{i}")
        for h in range(2):
            # --- load half-image (all 3 channels), flat row-blocks ---
            cht = chpool.tile([128, CH * HB * W], mybir.dt.float32, tag="ch")
            if do_in:
                # one [128,1024] DMA per (channel, row-block): each reads an
                # ascending-contiguous 512KB DRAM span (descriptor stream
                # stays sequential, which HBM likes under load)
                for c in range(CH):
                    for b in range(HB):
                        nc.sync.dma_start(
                            cht[:, (c * HB + b) * W : (c * HB + b + 1) * W],
                            x[i, c, (h * HB + b) * 128 : (h * HB + b + 1) * 128, :],
                        )
            if do_compute:
                c0 = cht[:, 0 : HB * W]
                c1 = cht[:, HB * W : 2 * HB * W]
                c2 = cht[:, 2 * HB * W : 3 * HB * W]
                nc.vector.tensor_tensor(c0, c0, c1, mybir.AluOpType.min)
                nc.vector.tensor_tensor(
                    dark[:, h * HB * W : (h + 1) * HB * W], c0, c2,
                    mybir.AluOpType.min,
                )

        ot = opool.tile([128, NB * W], mybir.dt.bfloat16, tag="out",
                        name=f"out{i}")
        if do_compute:
            # --- pass 1: 15-tap row sums -> t1[c, r] (transposed) ---
            t1 = t1pool.tile([128, NB * H], mybir.dt.bfloat16, tag="t1")

            def p1_lhsT(ob, sb, p0, p1, dark=dark):
                return dark[p0:p1, sb * W + ob * 128 : sb * W + ob * 128 + 128]

            def write_t1(bank, ps, ob, t1=t1):
                nc.scalar.activation(
                    t1[:, ob * H + bank * 512 : ob * H + bank * 512 + 512],
                    ps[:, :],
                    mybir.ActivationFunctionType.Copy,
                )

            _boxsum_pass(nc, pspool, p1_lhsT, write_t1, bmat)

            # --- pass 2: 15-tap col sums -> out[r, c]; scale, cast bf16 ---
            def p2_lhsT(ob, sb, p0, p1, t1=t1):
                return t1[p0:p1, sb * H + ob * 128 : sb * H + ob * 128 + 128]

            def write_out(bank, ps, ob, ot=ot, i=i):
                nc.scalar.activation(
                    ot[:, ob * W + bank * 512 : ob * W + bank * 512 + 512],
                    ps[:, :],
                    mybir.ActivationFunctionType.Copy,
                    scale=scale,
                )
                # store per block: overlaps pass-2 and keeps each write an
                # ascending-contiguous 256KB DRAM span
                if bank == 1 and do_out:
                    nc.sync.dma_start(
                        y[i, ob * 128 : (ob + 1) * 128, :],
                        ot[:, ob * W : (ob + 1) * W],
                    )

            _boxsum_pass(nc, pspool, p2_lhsT, write_out, bmat)
        elif do_out:
            nc.vector.memset(ot[:], 0.0)
            for ob in range(NB):
                nc.sync.dma_start(
                    y[i, ob * 128 : (ob + 1) * 128, :],
                    ot[:, ob * W : (ob + 1) * W],
                )


def kernel(images, weight):
    global LAST_RESULTS
    images = np.ascontiguousarray(np.asarray(images, dtype=np.float32))
    weight = np.asarray(weight, dtype=np.float64)
    # reference: conv with w = weight/225; weight is uniform (ones), so the
    # whole filter reduces to mean(weight)/225 * boxsum.
    scale = float(weight.mean()) / (KS * KS)

    if scale not in _PROGRAM_CACHE:
        _PROGRAM_CACHE[scale] = _build_program(scale)
    nc = _PROGRAM_CACHE[scale]
    bmat = _build_bmat()
    in_maps = [
        {
            "x": images[c * IMGS_PER_CORE : (c + 1) * IMGS_PER_CORE],
            "bmat": bmat,
        }
        for c in range(N_CORES)
    ]
    res = run_bass_kernel_spmd(nc, in_maps, core_ids=list(range(N_CORES)))
    LAST_RESULTS = res
    out = np.concatenate(
        [np.asarray(r["y"]).astype(np.float32)[:, None, :, :] for r in res.results],
        axis=0,
    )
    return out
